# revision 46
# baseline (speedup 1.0000x reference)
"""EGNN (gnn_message_passing) Trainium2 Bass kernel.

Model: 2-layer EGNN over B=16 molecules x N=256 nodes. Sharding:
data-parallel on batch, 2 molecules per core on 8 cores; tiny MLP
weights replicated.

Math restructuring (host-side weight prep):
- lipswish(x) = 0.909*silu(x): the 0.909 is folded into the *next*
  consumer's weights (edge_w2, gate_w, node_w1[m-cols], node_w2), so the
  device only computes silu/sigmoid.
- edge input e = [f_i, f_j, d_ij] @ W1 splits into:
    per-j term  W1[12:24].T @ f_j   -> K=12 matmul vs doubled featsT
    per-edge    W1[24] * d_ij       -> K=1 rank-1 vs rel_dist row
    per-i term  (W1[0:12].T @ f_i + b1) -> precomputed [50,256], applied
    as the per-partition *bias* of the silu activation.
- gate_w replicated to 64 columns so the gate matmul directly yields the
  [64, nedge] pre-broadcast gate (no separate broadcast matmul).
- rel_dist via ||xi||^2 + ||xj||^2 - 2 xi.xj with the Gram matrix on PE,
  packed into "row-pair" layout [128, 2*256] so each 512-edge chunk reads
  its distances as a single [1,512] row.
"""

import numpy as np

import concourse.bass as bass
import concourse.bacc as bacc
import concourse.mybir as mybir
from concourse.tile import TileContext
from concourse.bass_utils import run_bass_kernel_spmd

F32 = mybir.dt.float32
AF = mybir.ActivationFunctionType
ALU = mybir.AluOpType

LIP = 0.909
NCORES = 8
BM = 2            # molecules per core
N = 256           # nodes per molecule
L = 2             # layers
D = 12            # feature dim
M = 64            # message dim
EH = 50           # edge hidden
NI = D + M        # node-mlp input dim = 76
NCHUNK = N // 2   # 128 chunks of 512 edges (2 i-rows) per molecule-layer


def build_nc(variant="full"):
    # variant: "full" | "setup" (skip edge chunks) | "nodrow" (memset drow)
    nc = bacc.Bacc("TRN2", target_bir_lowering=False, debug=False)

    xT = nc.dram_tensor("xT", [BM, 6, N], F32, kind="ExternalInput")
    xnm = nc.dram_tensor("xnm", [BM, N, 6], F32, kind="ExternalInput")
    maskf = nc.dram_tensor("maskf", [BM, N], F32, kind="ExternalInput")
    ew1fi = nc.dram_tensor("ew1fi", [D, L * EH], F32, kind="ExternalInput")
    ew1fjd = nc.dram_tensor("ew1fjd", [D + 1, L * EH], F32,
                            kind="ExternalInput")
    ew2 = nc.dram_tensor("ew2", [EH, L * M], F32, kind="ExternalInput")
    gwr = nc.dram_tensor("gwr", [M, L * M], F32, kind="ExternalInput")
    eb1 = nc.dram_tensor("eb1", [EH, L], F32, kind="ExternalInput")
    eb2 = nc.dram_tensor("eb2", [M, L], F32, kind="ExternalInput")
    gb64 = nc.dram_tensor("gb64", [M, L], F32, kind="ExternalInput")
    lng = nc.dram_tensor("lng", [D, L], F32, kind="ExternalInput")
    lnb = nc.dram_tensor("lnb", [D, L], F32, kind="ExternalInput")
    nw1a = nc.dram_tensor("nw1a", [D, L * 24], F32, kind="ExternalInput")
    nw1b = nc.dram_tensor("nw1b", [M, L * 24], F32, kind="ExternalInput")
    nb1 = nc.dram_tensor("nb1", [24, L], F32, kind="ExternalInput")
    nw2 = nc.dram_tensor("nw2", [24, L * D], F32, kind="ExternalInput")
    nb2 = nc.dram_tensor("nb2", [D, L], F32, kind="ExternalInput")
    mw1 = nc.dram_tensor("mw1", [D, M], F32, kind="ExternalInput")
    mb1 = nc.dram_tensor("mb1", [M, 1], F32, kind="ExternalInput")
    mw2 = nc.dram_tensor("mw2", [M, 2], F32, kind="ExternalInput")
    mb2 = nc.dram_tensor("mb2", [2, 1], F32, kind="ExternalInput")
    out = nc.dram_tensor("out", [BM, N, 2, 6], F32, kind="ExternalOutput")

    with TileContext(nc) as tc:
        with (
            tc.tile_pool(name="singles", bufs=1) as S,
            tc.tile_pool(name="mol", bufs=2) as MP,
            tc.tile_pool(name="lay", bufs=2) as LP,
            tc.tile_pool(name="stream", bufs=2) as ST,
            tc.tile_pool(name="psum", bufs=2, space="PSUM") as PS,
        ):
            # ---- load all weights into SBUF once ----
            def ld(dram, p, f, nm):
                t = S.tile([p, f], F32, tag=nm, name=nm)
                nc.sync.dma_start(out=t, in_=dram[:, :])
                return t

            ew1fi_s = ld(ew1fi, D, L * EH, "w_ew1fi")
            ew1fjd_s = ld(ew1fjd, D + 1, L * EH, "w_ew1fjd")
            ew2_s = ld(ew2, EH, L * M, "w_ew2")
            gwr_s = ld(gwr, M, L * M, "w_gwr")
            eb1_s = ld(eb1, EH, L, "w_eb1")
            eb2_s = ld(eb2, M, L, "w_eb2")
            gb64_s = ld(gb64, M, L, "w_gb64")
            lng_s = ld(lng, D, L, "w_lng")
            lnb_s = ld(lnb, D, L, "w_lnb")
            nw1a_s = ld(nw1a, D, L * 24, "w_nw1a")
            nw1b_s = ld(nw1b, M, L * 24, "w_nw1b")
            nb1_s = ld(nb1, 24, L, "w_nb1")
            nw2_s = ld(nw2, 24, L * D, "w_nw2")
            nb2_s = ld(nb2, D, L, "w_nb2")
            mw1_s = ld(mw1, D, M, "w_mw1")
            mb1_s = ld(mb1, M, 1, "w_mb1")
            mw2_s = ld(mw2, M, 2, "w_mw2")
            mb2_s = ld(mb2, 2, 1, "w_mb2")

            ones_r = S.tile([1, 128], F32)      # row of ones (K=1 lhsT)
            nc.vector.memset(ones_r, 1.0)
            ones_c = S.tile([128, 1], F32)      # column of ones
            nc.vector.memset(ones_c, 1.0)
            c12 = S.tile([D, 1], F32)           # 1/12 column (LN mean)
            nc.vector.memset(c12, 1.0 / D)
            eps = S.tile([1, 1], F32)
            nc.vector.memset(eps, 1e-5)

            # output staging: [2 ch, 256 n, 6 pad] zero except k=0 slots
            opad = S.tile([2, N, 6], F32)
            nc.vector.memset(opad, 0.0)

            for mol in range(BM):
                # ---- per-molecule setup ----
                xT_s = MP.tile([6, N], F32, tag="xT")
                nc.sync.dma_start(out=xT_s, in_=xT[mol])
                xnm_s = MP.tile([128, 2, 6], F32, tag="xnm")
                nc.sync.dma_start(
                    out=xnm_s, in_=xnm[mol].rearrange("(b p) f -> p b f", p=128)
                )

                feats = MP.tile([D, N], F32, tag="feats")
                nc.sync.dma_start(out=feats[0:6, :], in_=xT[mol])
                nc.sync.dma_start(out=feats[6:12, :], in_=xT[mol])

                xT2n = MP.tile([6, N], F32, tag="xT2n")  # -2x (feature-major)
                nc.vector.tensor_scalar_mul(xT2n, xT_s, -2.0)

                sq6 = MP.tile([6, N], F32, tag="sq6")
                nc.vector.tensor_mul(sq6, xT_s, xT_s)
                ps_nr = PS.tile([1, N], F32, tag="ps")
                nc.tensor.matmul(ps_nr, lhsT=ones_c[0:6, :], rhs=sq6,
                                 start=True, stop=True)
                nsq_row = MP.tile([1, N], F32, tag="nsq_row")
                nc.vector.tensor_copy(out=nsq_row, in_=ps_nr)

                sqn = MP.tile([128, 12], F32, tag="sqn")  # x^2 node-major
                nc.vector.tensor_mul(
                    sqn, xnm_s.rearrange("p b f -> p (b f)"),
                    xnm_s.rearrange("p b f -> p (b f)"))
                nsq_col = MP.tile([128, 2], F32, tag="nsq_col")
                nc.vector.reduce_sum(
                    nsq_col, sqn.rearrange("p (b f) -> p b f", b=2),
                    axis=mybir.AxisListType.X)

                # rel_dist, natural [i, j] layout in two 128-row blocks
                rd_blk = []
                for blk in range(2):
                    ps_rd = PS.tile([128, N], F32, tag="ph")
                    nc.tensor.matmul(ps_rd, lhsT=xT2n[:, blk * 128:(blk + 1) * 128],
                                     rhs=xT_s, start=True, stop=False)
                    nc.tensor.matmul(ps_rd, lhsT=ones_r, rhs=nsq_row,
                                     start=False, stop=True)
                    rd_sb = MP.tile([128, N], F32, tag=f"rd{blk}",
                                    name=f"rd{blk}")
                    nc.vector.tensor_scalar_add(rd_sb, ps_rd,
                                                nsq_col[:, blk:blk + 1])
                    rd_blk.append(rd_sb)

                mask_rep = MP.tile([D, N], F32, tag="mask_rep")
                mrow = maskf[mol:mol + 1, :]
                nc.sync.dma_start(
                    out=mask_rep,
                    in_=bass.AP(tensor=mrow.tensor, offset=mrow.offset,
                                ap=[[0, D]] + list(mrow.ap[1:])))

                for lay in range(L):
                    w1fi = ew1fi_s[:, lay * EH:(lay + 1) * EH]
                    w1fjd = ew1fjd_s[:, lay * EH:(lay + 1) * EH]
                    w2 = ew2_s[:, lay * M:(lay + 1) * M]
                    gw = gwr_s[:, lay * M:(lay + 1) * M]

                    # per-i bias FiWb[50, 256] = W1fi.T @ feats + b1
                    ps_fi = PS.tile([EH, N], F32, tag="ph")
                    nc.tensor.matmul(ps_fi, lhsT=w1fi, rhs=feats,
                                     start=True, stop=True)
                    fiwb = LP.tile([EH, N], F32, tag="fiwb")
                    nc.vector.tensor_scalar_add(fiwb, ps_fi,
                                                eb1_s[:, lay:lay + 1])

                    # doubled feats for the fj term
                    fj2 = LP.tile([D, 2 * N], F32, tag="fj2")
                    nc.gpsimd.tensor_copy(out=fj2[:, 0:N], in_=feats)
                    nc.gpsimd.tensor_copy(out=fj2[:, N:2 * N], in_=feats)

                    # aggregated messages, written column-wise per chunk
                    magg = LP.tile([M, N], F32, tag="magg")

                    # ---- edge chunks, supergrouped G=4 (2048 edges) ----
                    # Per chunk: one K=13 matmul ([fj; d] stacked operand),
                    # W2 matmul, replicated-gate matmul; pre-activations are
                    # staged into wide SBUF tiles by DVE (which also adds the
                    # per-i fi bias), so ACT runs 3 wide ops per group.
                    G = 4
                    W = G * 512
                    nchunk = {"setup": 0, "tiny": G}.get(variant, NCHUNK)
                    if nchunk < NCHUNK:
                        nc.vector.memset(magg, 0.0)
                    for grp in range(nchunk // G):
                        hw_pre = ST.tile([EH, W], F32, tag="hw_pre")
                        for q in range(G):
                            c = grp * G + q
                            i0 = 2 * c
                            e13 = ST.tile([D + 1, 512], F32, tag="e13")
                            nc.gpsimd.tensor_copy(out=e13[0:D, :], in_=fj2)
                            rdb = rd_blk[c // 64]
                            r = (c % 64) * 2
                            nc.sync.dma_start(out=e13[D:D + 1, 0:N],
                                              in_=rdb[r:r + 1, :])
                            nc.sync.dma_start(out=e13[D:D + 1, N:2 * N],
                                              in_=rdb[r + 1:r + 2, :])
                            ps_h = PS.tile([EH, 512], F32, tag="ph")
                            nc.tensor.matmul(ps_h, lhsT=w1fjd, rhs=e13,
                                             start=True, stop=True)
                            for half in range(2):
                                nc.vector.tensor_scalar_add(
                                    hw_pre[:, q * 512 + half * N:
                                           q * 512 + (half + 1) * N],
                                    ps_h[:, half * N:(half + 1) * N],
                                    fiwb[:, i0 + half:i0 + half + 1])
                        h_w = ST.tile([EH, W], F32, tag="h_w")
                        nc.scalar.activation(h_w, hw_pre, AF.Silu)
                        mp_w = ST.tile([M, W], F32, tag="mp_w")
                        for q in range(G):
                            ps_m = PS.tile([M, 512], F32, tag="pm")
                            nc.tensor.matmul(ps_m, lhsT=w2,
                                             rhs=h_w[:, q * 512:(q + 1) * 512],
                                             start=True, stop=True)
                            nc.vector.tensor_copy(
                                out=mp_w[:, q * 512:(q + 1) * 512], in_=ps_m)
                        m_w = ST.tile([M, W], F32, tag="m_w")
                        nc.scalar.activation(m_w, mp_w, AF.Silu,
                                             bias=eb2_s[:, lay:lay + 1])
                        sp_w = ST.tile([M, W], F32, tag="sp_w")
                        for q in range(G):
                            ps_s = PS.tile([M, 512], F32, tag="ps")
                            nc.tensor.matmul(ps_s, lhsT=gw,
                                             rhs=m_w[:, q * 512:(q + 1) * 512],
                                             start=True, stop=True)
                            nc.vector.tensor_copy(
                                out=sp_w[:, q * 512:(q + 1) * 512], in_=ps_s)
                        g_w = ST.tile([M, W], F32, tag="g_w")
                        nc.scalar.activation(g_w, sp_w, AF.Sigmoid,
                                             bias=gb64_s[:, lay:lay + 1])
                        sc = ST.tile([M, W], F32, tag="sc")
                        for q in range(G):
                            i0 = 2 * (grp * G + q)
                            for half in range(2):
                                sl = slice(q * 512 + half * N,
                                           q * 512 + (half + 1) * N)
                                nc.vector.scalar_tensor_tensor(
                                    out=sc[:, sl], in0=m_w[:, sl], scalar=1.0,
                                    in1=g_w[:, sl],
                                    op0=ALU.bypass, op1=ALU.mult,
                                    accum_out=magg[:, i0 + half:i0 + half + 1])

                    # ---- LayerNorm on feats -> ni[0:12] ----
                    sqf = LP.tile([D, N], F32, tag="sqf")
                    nc.vector.tensor_mul(sqf, feats, feats)
                    ps_mu = PS.tile([1, N], F32, tag="ps")
                    nc.tensor.matmul(ps_mu, lhsT=c12, rhs=feats,
                                     start=True, stop=True)
                    ps_ms = PS.tile([1, N], F32, tag="pm")
                    nc.tensor.matmul(ps_ms, lhsT=c12, rhs=sqf,
                                     start=True, stop=True)
                    stat = LP.tile([1, 2 * N], F32, tag="stat")
                    # stat[0:256] = mean; compute var -> rstd into stat[256:]
                    nc.vector.tensor_copy(out=stat[:, 0:N], in_=ps_mu)
                    musq = LP.tile([1, N], F32, tag="musq")
                    nc.vector.tensor_mul(musq, stat[:, 0:N], stat[:, 0:N])
                    var = LP.tile([1, N], F32, tag="var")
                    nc.vector.tensor_sub(var, ps_ms, musq)
                    sd = LP.tile([1, N], F32, tag="sd")
                    nc.scalar.activation(sd, var, AF.Sqrt, bias=eps[:, 0:1])
                    nc.vector.reciprocal(stat[:, N:2 * N], sd)
                    ps_b = PS.tile([D, 2 * N], F32, tag="pg")
                    nc.tensor.matmul(ps_b, lhsT=ones_r[:, 0:D], rhs=stat,
                                     start=True, stop=True)
                    ctr = LP.tile([D, N], F32, tag="ctr")
                    nc.vector.tensor_sub(ctr, feats, ps_b[:, 0:N])
                    nrm = LP.tile([D, N], F32, tag="nrm")
                    nc.vector.tensor_mul(nrm, ctr, ps_b[:, N:2 * N])
                    normed = LP.tile([D, N], F32, tag="normed")
                    nc.vector.tensor_scalar(
                        out=normed, in0=nrm,
                        scalar1=lng_s[:, lay:lay + 1],
                        scalar2=lnb_s[:, lay:lay + 1],
                        op0=ALU.mult, op1=ALU.add)

                    # ---- node MLP + residual -> new feats ----
                    ps_z1 = PS.tile([24, N], F32, tag="ph")
                    nc.tensor.matmul(ps_z1,
                                     lhsT=nw1a_s[:, lay * 24:(lay + 1) * 24],
                                     rhs=normed, start=True, stop=False)
                    nc.tensor.matmul(ps_z1,
                                     lhsT=nw1b_s[:, lay * 24:(lay + 1) * 24],
                                     rhs=magg, start=False, stop=True)
                    s1 = LP.tile([24, N], F32, tag="s1")
                    nc.scalar.activation(s1, ps_z1, AF.Silu,
                                         bias=nb1_s[:, lay:lay + 1])
                    ps_z2 = PS.tile([D, N], F32, tag="pm")
                    nc.tensor.matmul(ps_z2,
                                     lhsT=nw2_s[:, lay * D:(lay + 1) * D],
                                     rhs=s1, start=True, stop=True)
                    feats_new = MP.tile([D, N], F32, tag="feats")
                    nc.vector.scalar_tensor_tensor(
                        out=feats_new, in0=ps_z2,
                        scalar=nb2_s[:, lay:lay + 1], in1=feats,
                        op0=ALU.add, op1=ALU.add)
                    feats = feats_new

                # ---- final head ----
                fmask = MP.tile([D, N], F32, tag="fmask")
                nc.vector.tensor_mul(fmask, feats, mask_rep)
                ps_r = PS.tile([M, N], F32, tag="ph")
                nc.tensor.matmul(ps_r, lhsT=mw1_s, rhs=fmask,
                                 start=True, stop=True)
                r1 = MP.tile([M, N], F32, tag="r1")
                nc.scalar.activation(r1, ps_r, AF.Relu, bias=mb1_s[:, 0:1])
                ps_o = PS.tile([2, N], F32, tag="pm")
                nc.tensor.matmul(ps_o, lhsT=mw2_s, rhs=r1,
                                 start=True, stop=True)
                nc.vector.tensor_scalar_add(opad[:, :, 0:1], ps_o,
                                            mb2_s[:, 0:1])
                nc.sync.dma_start(
                    out=out[mol].rearrange("n c k -> c n k"), in_=opad)

    nc.finalize()
    return nc


_NC = None


def _get_nc():
    global _NC
    if _NC is None:
        _NC = build_nc()
    return _NC


def _prep_maps(x, mask, edge_w1, edge_b1, edge_w2, edge_b2, gate_w, gate_b,
               ln_g, ln_b, node_w1, node_b1, node_w2, node_b2,
               mlp_w1, mlp_b1, mlp_w2, mlp_b2):
    f = np.float32
    x = np.asarray(x, f)
    B = x.shape[0]
    maskf = np.asarray(mask, f)

    # host-side weight layout + lipswish constant folding
    ew1_t = np.transpose(np.asarray(edge_w1, f), (1, 0, 2))  # [25, L, 50]
    ew1fi_h = np.ascontiguousarray(ew1_t[0:12].reshape(D, L * EH))
    ew1fjd_h = np.ascontiguousarray(ew1_t[12:25].reshape(D + 1, L * EH))
    ew2_h = np.ascontiguousarray(
        np.transpose(np.asarray(edge_w2, f) * LIP, (1, 0, 2)).reshape(EH, L * M))
    gw_rep = np.repeat(np.asarray(gate_w, f) * LIP, M, axis=2)  # [L,64,64]
    gwr_h = np.ascontiguousarray(np.transpose(gw_rep, (1, 0, 2)).reshape(M, L * M))
    eb1_h = np.ascontiguousarray(np.asarray(edge_b1, f).T)
    eb2_h = np.ascontiguousarray(np.asarray(edge_b2, f).T)
    gb64_h = np.ascontiguousarray(
        np.repeat(np.asarray(gate_b, f), M, axis=1).T)     # [64, L]
    lng_h = np.ascontiguousarray(np.asarray(ln_g, f).T)
    lnb_h = np.ascontiguousarray(np.asarray(ln_b, f).T)
    nw1_f = np.asarray(node_w1, f).copy()
    nw1_f[:, D:, :] *= LIP          # fold lipswish of aggregated messages
    nw1_t = np.transpose(nw1_f, (1, 0, 2))                  # [76, L, 24]
    nw1a_h = np.ascontiguousarray(nw1_t[0:D].reshape(D, L * 24))
    nw1b_h = np.ascontiguousarray(nw1_t[D:].reshape(M, L * 24))
    nb1_h = np.ascontiguousarray(np.asarray(node_b1, f).T)
    nw2_h = np.ascontiguousarray(
        np.transpose(np.asarray(node_w2, f) * LIP, (1, 0, 2)).reshape(24, L * D))
    nb2_h = np.ascontiguousarray(np.asarray(node_b2, f).T)
    mw1_h = np.asarray(mlp_w1, f)
    mb1_h = np.asarray(mlp_b1, f).reshape(M, 1)
    mw2_h = np.asarray(mlp_w2, f)
    mb2_h = np.asarray(mlp_b2, f).reshape(2, 1)

    shared = dict(ew1fi=ew1fi_h, ew1fjd=ew1fjd_h,
                  ew2=ew2_h, gwr=gwr_h, eb1=eb1_h, eb2=eb2_h,
                  gb64=gb64_h, lng=lng_h, lnb=lnb_h, nw1a=nw1a_h,
                  nw1b=nw1b_h, nb1=nb1_h,
                  nw2=nw2_h, nb2=nb2_h, mw1=mw1_h, mb1=mb1_h, mw2=mw2_h,
                  mb2=mb2_h)

    in_maps = []
    for core in range(NCORES):
        xs = x[core * BM:(core + 1) * BM]                  # [2, 256, 6]
        in_maps.append(dict(
            xT=np.ascontiguousarray(np.transpose(xs, (0, 2, 1))),
            xnm=np.ascontiguousarray(xs),
            maskf=np.ascontiguousarray(maskf[core * BM:(core + 1) * BM]),
            **{k: v.copy() for k, v in shared.items()},
        ))
    return in_maps


def kernel(**inputs):
    nc = _get_nc()
    in_maps = _prep_maps(**inputs)
    res = run_bass_kernel_spmd(nc, in_maps, core_ids=list(range(NCORES)))
    out = np.concatenate([r["out"] for r in res.results], axis=0)
    return out.astype(np.float32)
